# revision 42
# baseline (speedup 1.0000x reference)
"""Attention-score softmax kernel for Trainium2 (8 NeuronCores, SPMD).

reference:
    energies = history @ W.T + b          # [seq, hid]
    scores   = out_state @ energies.T     # [state, seq]
    out      = softmax(scores, axis=-1)

Algebra: scores = out_state @ W @ history.T + (out_state @ b) 1^T; the bias
is constant along rows so it drops out of the row softmax.  Two chained
GEMMs per core (rows of out_state sharded 8 ways, W/history replicated):
    T      = out_state_c @ W               # [256, hid]
    scores = T @ history.T                 # [256, seq]

All matmuls run as fp8e4 (e4m3) DoubleRow pairs -- 2 contraction slices per
instruction at 0.5 cyc/row, 4x the fp16 MAC rate.  Accuracy comes from a
residual ("split") representation of every operand:  x ~ hi + lo with
hi = e4m3(sx), lo = e4m3(sx - hi), s a power-of-2 scale chosen so the
residual stays in fp8 normal range (s=4 for out_state/history/T, s=64 for
W, whose 1/sqrt(h) magnitude would push residuals into subnormals).
Chains computed per GEMM:
    MM1: hi@hi + lo@hi + hi@lo + lo@lo          (4 chains, exact pair product)
    MM2: hi@hi + lo@hi + hi@lo [+ lo@lo on 4/8 kpairs]
The partial MM2 4th chain exists to keep the PE busier than the history
DMA stream cadence: the cost model halves the PE clock for ~3us after any
idle gap, so every phase is sized to be (slightly) PE-paced.

T is split on-chip straight out of PSUM (= 256*T):  ACT writes
t_hi = e4m3(psum/64) and DVE writes t_lo = e4m3(psum/64 - t_hi); drains
run bank-by-bank during MM1's last k-pair so MM2 starts without a bubble.

MM2 PSUM holds 16*scores; the softmax exp fuses the 1/16 descale
(exp(psum/16 - m) on ACT with the row-sum via accum_out).  probs staged
fp16 (2x DVE rescale rate, half the output DMA bytes).

The kernel is input-DMA-bound (26.2MB/core at the 360GB/s DMA ceiling =
72.8us), so the tail after the last input byte is minimized: the final
512 seq columns are a 384-col + 128-col piece with independently
streamed hi/lo data, ordered so i-tile 0's stats + softmax correction
(corr_s = exp(m_s-M), Z via a fused multiply-accumulate) complete while
i-tile 1 is still on the PE, and the 2.1MB fp16 output streams out in
1024-col chunks behind the rescales (DVE 2/3, ACT 1/3).
"""

import contextlib

import numpy as np

STATE, SEQ, HID, NCORES = 2048, 4096, 2048, 8
IS = STATE // NCORES   # 256 out_state rows per core
KP = HID // 256        # 8 e-kpairs (256-deep DoubleRow contraction) for MM1
HP = HID // 256        # 8 h-kpairs for MM2
NSLAB = SEQ // 512     # 8 j slabs of 512
NI = IS // 128         # 2 output partition tiles

NWARM = 24             # PE warm-up matmuls (p-state ramp cover)
# MM2 column blocks: the final 512 splits into 384+128 so almost no PE
# work remains once the last h_lo bytes land, and the softmax ladder +
# output DMA start as early as possible.
SLABS = [(s * 512, 512, s) for s in range(NSLAB - 1)]
SLABS += [(3584, 384, 7), (3968, 128, 8)]   # hs 7/8 = last-slab sub-tiles
NST = len(SLABS)       # 9 stat columns

TRACE = False
_CACHE: dict = {}


def _build():
    import concourse.bacc as bacc
    import concourse.mybir as mybir
    import concourse.tile as tile

    f16, f32, f8 = mybir.dt.float16, mybir.dt.float32, mybir.dt.float8e4
    X = mybir.AxisListType.X
    Alu = mybir.AluOpType
    Exp = mybir.ActivationFunctionType.Exp
    Copy = mybir.ActivationFunctionType.Copy
    DR = mybir.MatmulPerfMode.DoubleRow

    nc = bacc.Bacc("TRN2", target_bir_lowering=False, debug=False)
    # layouts (host-prepped, fp8e4):
    #  s_*: [128, KP*2*IS]    = [p, k, d, i], e = k*256 + d*128 + p
    #  w_*: [128, KP*2*HID]   = [p, k, d, h]
    #  h_*: [128, NSLAB*HP*2*512] = [p, s, q, d, j], h = q*256 + d*128 + p
    s_hi = nc.dram_tensor("s_hi", [128, KP * 2 * IS], f8, kind="ExternalInput")
    s_lo = nc.dram_tensor("s_lo", [128, KP * 2 * IS], f8, kind="ExternalInput")
    w_hi = nc.dram_tensor("w_hi", [128, KP * 2 * HID], f8, kind="ExternalInput")
    w_lo = nc.dram_tensor("w_lo", [128, KP * 2 * HID], f8, kind="ExternalInput")
    h_hi = nc.dram_tensor("h_hi", [128, NSLAB * HP * 2 * 512], f8,
                          kind="ExternalInput")
    h_lo = nc.dram_tensor("h_lo", [128, NSLAB * HP * 2 * 512], f8,
                          kind="ExternalInput")
    out = nc.dram_tensor("out", [IS, SEQ], f16, kind="ExternalOutput")

    with tile.TileContext(nc) as tc:
        with (
            tc.tile_pool(name="res", bufs=1) as res,
            tc.tile_pool(name="wstream", bufs=6) as wstream,
            tc.tile_pool(name="hstream", bufs=4) as hstream,
            tc.tile_pool(name="psum", bufs=8, space="PSUM") as psum,
        ):
            # ---- input DMA (issue order == DMA_ENGINES service order) ----
            st_hi = res.tile([128, KP, 2, IS], f8, tag="sthi", name="st_hi")
            st_lo = res.tile([128, KP, 2, IS], f8, tag="stlo", name="st_lo")
            wch_hi = [wstream.tile([128, 2, HID], f8, tag="wh", name=f"wh{k}")
                      for k in range(KP)]
            wch_lo = [wstream.tile([128, 2, HID], f8, tag="wl", name=f"wl{k}")
                      for k in range(KP)]
            # the last slab is stored/loaded as two independent j-pieces
            # (384 + 128 cols) so its hi/lo data (the final input bytes)
            # lands piecewise: only a 0.26MB lo quarter and 0.2us of PE
            # stand between the last byte and the softmax ladder
            hist_hi = [hstream.tile([128, HP, 2, 512], f8, tag="hh",
                                    name=f"hh{s}") for s in range(NSLAB - 1)]
            hist_lo = [hstream.tile([128, HP, 2, 512], f8, tag="hl",
                                    name=f"hl{s}") for s in range(NSLAB - 1)]
            for w in (384, 128):
                hist_hi.append(hstream.tile([128, HP, 2, w], f8, tag="hh",
                                            name=f"hh7_{w}"))
                hist_lo.append(hstream.tile([128, HP, 2, w], f8, tag="hl",
                                            name=f"hl7_{w}"))

            WC, HC = 2 * HID, HP * 2 * 512
            nc.sync.dma_start(wch_hi[0][:], w_hi[:, 0:WC])
            nc.sync.dma_start(st_hi[:], s_hi[:, :])
            nc.sync.dma_start(st_lo[:], s_lo[:, :])
            nc.sync.dma_start(wch_lo[0][:], w_lo[:, 0:WC])
            for k in range(1, KP):
                nc.sync.dma_start(wch_hi[k][:], w_hi[:, k * WC:(k + 1) * WC])
                nc.sync.dma_start(wch_lo[k][:], w_lo[:, k * WC:(k + 1) * WC])
            for s in range(NSLAB - 1):
                nc.sync.dma_start(hist_hi[s][:], h_hi[:, s * HC:(s + 1) * HC])
                nc.sync.dma_start(hist_lo[s][:], h_lo[:, s * HC:(s + 1) * HC])
            # last slab: hi/lo per j-piece, each piece fully delivered
            # before the next so tail piece 7 is done while 8 is in flight
            c7 = (NSLAB - 1) * HC
            for k, w in ((7, 384), (8, 128)):
                wc = HP * 2 * w
                nc.sync.dma_start(hist_hi[k][:], h_hi[:, c7:c7 + wc])
                nc.sync.dma_start(hist_lo[k][:], h_lo[:, c7:c7 + wc])
                c7 += wc

            # ---- PE warm-up: cover the p-state ramp + first-chunk DMA ----
            warm = res.tile([128, 128], f8, tag="warm", name="warm")
            nc.vector.memset(warm[:], 0.0)
            pwarm = psum.tile([128, 2, 256], f32, tag="ps", name="pwarm")
            for d in range(NWARM):
                nc.tensor.matmul(
                    pwarm[:, 0, 0:128], warm[:], warm[:],
                    start=(d == 0), stop=(d == NWARM - 1),
                )

            # ---- MM1: psum bank j = 256*T.T[h-pair j] as [128, d, i] ----
            ps1 = [psum.tile([128, 2, 256], f32, tag="ps", name=f"ps1_{j}")
                   for j in range(HP)]
            t_hi = [res.tile([128, 2, 256], f8, tag=f"thi{j}", name=f"thi{j}")
                    for j in range(HP)]
            t_lo = [res.tile([128, 2, 256], f8, tag=f"tlo{j}", name=f"tlo{j}")
                    for j in range(HP)]

            def mm1(k, ci, j, d, start, stop):
                ss = st_hi if ci in (0, 2) else st_lo
                ww = wch_hi[k] if ci in (0, 1) else wch_lo[k]
                hcol = (2 * j + d) * 128
                nc.tensor.matmul(
                    ps1[j][:, d, :],
                    ww[:, :, hcol:hcol + 128],   # lhsT [e,(2),h]
                    ss[:, k, :, :],              # rhs  [e,(2),i]
                    start=start, stop=stop, perf_mode=DR,
                )

            def drain(j):
                # t_hi = e4m3(psum/64) = e4m3(4*T); t_lo = residual
                nc.scalar.activation(t_hi[j][:], ps1[j][:], Copy,
                                     bias=0.0, scale=1.0 / 64)
                nc.vector.scalar_tensor_tensor(
                    t_lo[j][:], ps1[j][:], 1.0 / 64, t_hi[j][:],
                    op0=Alu.mult, op1=Alu.subtract,
                )

            for k in range(KP - 1):
                for ci in range(4):
                    for j in range(HP):
                        for d in range(2):
                            mm1(k, ci, j, d,
                                start=(k == 0 and ci == 0 and d == 0),
                                stop=False)
            # last k-pair: bank-major so drains overlap remaining matmuls
            for j in range(HP):
                for ci in range(4):
                    for d in range(2):
                        mm1(KP - 1, ci, j, d, start=False,
                            stop=(ci == 3 and d == 1))
                drain(j)

            # ---- MM2 + online softmax (per-slab max, end corrections) ----
            probs, negq16, negqs, sums = [], [], [], []
            for i in range(NI):
                probs.append(res.tile([128, SEQ], f16, tag=f"probs{i}",
                                      name=f"probs{i}"))
                negq16.append(res.tile([128, NST], f32, tag=f"nq16{i}",
                                       name=f"nq16{i}"))
                negqs.append(res.tile([128, NST], f32, tag=f"nqs{i}",
                                      name=f"nqs{i}"))
                sums.append(res.tile([128, NST], f32, tag=f"sums{i}",
                                     name=f"sums{i}"))

            # chain-outer: both h_hi chains first, so each tile's compute
            # can start as soon as h_hi[s] lands (h_lo[s] still in flight)
            CH2 = [(t_hi, hist_hi), (t_lo, hist_hi), (t_hi, hist_lo)]

            def mm2_chains(p2, si, i, chains, start, stop):
                col0, width, hs = SLABS[si]
                j0 = 0 if hs >= NSLAB - 1 else col0 - hs * 512
                n = len(chains) * HP
                m = 0
                for tt_, hh_ in chains:
                    for q in range(HP):
                        nc.tensor.matmul(
                            p2[:],
                            tt_[q][:, :, i * 128:(i + 1) * 128],
                            hh_[hs][:, q, :, j0:j0 + width],
                            start=(start and m == 0),
                            stop=(stop and m == n - 1),
                            perf_mode=DR,
                        )
                        m += 1

            def stats(p2, si, i, prio=False):
                col0, width, hs = SLABS[si]
                ctx = tc.high_priority() if prio else contextlib.nullcontext()
                with ctx:
                    nc.vector.reduce_max(negq16[i][:, si:si + 1], p2[:],
                                         axis=X, negate=True)
                    nc.vector.tensor_scalar_mul(
                        negqs[i][:, si:si + 1], negq16[i][:, si:si + 1],
                        1.0 / 16)
                    nc.scalar.activation(
                        probs[i][:, col0:col0 + width],
                        p2[:],
                        Exp,
                        bias=negqs[i][:, si:si + 1],
                        scale=1.0 / 16,
                        accum_out=sums[i][:, si:si + 1],
                    )

            for si in range(NST - 2):
                for i in range(NI):
                    p2 = psum.tile([128, SLABS[si][1]], f32, tag="ps",
                                   name=f"ps2_{si}_{i}")
                    mm2_chains(p2, si, i, CH2, start=True, stop=True)
                    stats(p2, si, i)

            # ---- tail: the two final 128-wide pieces per i-tile.  All
            # h_hi-dependent chains for every piece go first; the h_lo
            # chains (gated on the very last DMA bytes) follow, so ~0.8us
            # of PE work remains once h_lo[7] lands.  The i0 softmax ladder
            # + rescales + output DMA are emitted before i1's lo-chains so
            # the in-order DVE/ACT queues pipeline the tail. ----
            out16 = [res.tile([128, SEQ], f16, tag=f"out16_{i}",
                              name=f"out16_{i}") for i in range(NI)]

            def ladder(i):
                # corr_s = exp(m_s - M); Z = sum corr*sums; f = corr/Z.
                # DVE rescales use (c, iv) two-scalar form; f only feeds the
                # ACT rescales, so it is computed last.
                negM = res.tile([128, 1], f32, tag=f"negM{i}", name=f"negM{i}")
                nc.vector.tensor_reduce(out=negM[:], in_=negqs[i][:], axis=X,
                                        op=Alu.min)
                c = res.tile([128, NST], f32, tag=f"corr{i}", name=f"corr{i}")
                nc.scalar.activation(c[:], negqs[i][:], Exp,
                                     bias=negM[:, 0:1], scale=-1.0)
                zp = res.tile([128, NST], f32, tag=f"zp{i}", name=f"zp{i}")
                z = res.tile([128, 1], f32, tag=f"z{i}", name=f"z{i}")
                nc.vector.scalar_tensor_tensor(zp[:], sums[i][:], 1.0, c[:],
                                               op0=Alu.mult, op1=Alu.mult,
                                               accum_out=z[:])
                iv = res.tile([128, 1], f32, tag=f"inv{i}", name=f"inv{i}")
                nc.vector.reciprocal(iv[:], z[:])
                fi = res.tile([128, NST], f32, tag=f"f{i}", name=f"f{i}")
                nc.vector.tensor_scalar_mul(fi[:], c[:], iv[:, 0:1])
                return c, iv, fi

            # tail pieces = the last 512 columns (384+128) x both i.
            # All h_hi chains first (PE runs them while h_lo[7] is in
            # flight); then lo-chains + stats i0-first; ladders before any
            # rescale so neither engine queue blocks the other i's stats.
            # i0 runs completely first (hi, lo, stats, ladder) so its output
            # rescales + DMA stream overlap i1's chains still on the PE.
            # Rescales: DVE (2x fp16, fused c*inv two-scalar) does 2/3,
            # ACT every 3rd piece (via f); 1024-col DMA chunks keep the out
            # stream transfer-paced (SP issue is 565ns/DMA).
            fs = {}
            pend = {0: 0, 1: 0}

            def rescale(i, sis):
                c, iv, fi = fs[i]
                for si in sis:
                    col0, width, _ = SLABS[si]
                    sl = slice(col0, col0 + width)
                    # ACT takes mid-stream pieces: their transfer slots come
                    # late enough that ACT's slower muls stay off the DMA
                    # stream's critical path
                    if si in (2, 4, 6):
                        nc.scalar.mul(out16[i][:, sl], probs[i][:, sl],
                                      mul=fi[:, si:si + 1])
                    else:
                        nc.vector.tensor_scalar(
                            out16[i][:, sl], probs[i][:, sl],
                            c[:, si:si + 1], iv[:, 0:1],
                            op0=Alu.mult, op1=Alu.mult)
                    pend[i] += width
                    # i0's first chunk ships at 512: it opens the out stream
                    # without waiting for si1's rescale behind i1's stats
                    lim = 512 if (i == 0 and si == 0) else 1024
                    if pend[i] >= lim or si == NST - 1:
                        dsl = slice(col0 + width - pend[i], col0 + width)
                        nc.sync.dma_start(out[i * 128:(i + 1) * 128, dsl],
                                          out16[i][:, dsl])
                        pend[i] = 0

            tps = (NST - 2, NST - 1)

            def piece(si, i):       # hi-chains, lo-chains, stats for one tile
                p2 = psum.tile([128, SLABS[si][1]], f32, tag="ps",
                               name=f"ps2_{si}_{i}")
                mm2_chains(p2, si, i, CH2[:2], start=True, stop=False)
                mm2_chains(p2, si, i, CH2[2:], start=False, stop=True)
                stats(p2, si, i, prio=True)

            for si in tps:          # i0: 7a completes before 7b's data lands
                piece(si, 0)
            with tc.high_priority():
                fs[0] = ladder(0)
                rescale(0, range(0, 4))
            for si in tps:
                piece(si, 1)
            rescale(0, range(4, NST))
            with tc.high_priority():
                fs[1] = ladder(1)
            rescale(1, range(0, NST))

    nc.finalize()
    return nc


def _split(x, scale, e4):
    hi = (x * scale).astype(e4)
    lo = ((x * scale) - hi.astype(np.float32)).astype(e4)
    return hi, lo


def kernel(**inputs: np.ndarray) -> np.ndarray:
    import ml_dtypes
    from concourse.bass_utils import run_bass_kernel_spmd

    e4 = ml_dtypes.float8_e4m3
    out_state = np.asarray(inputs["out_state"], dtype=np.float32)
    history = np.asarray(inputs["history"], dtype=np.float32)
    W = np.asarray(inputs["W"], dtype=np.float32)
    # inputs["b"] intentionally unused: softmax(x + c 1^T) == softmax(x).

    if "nc" not in _CACHE:
        _CACHE["nc"] = _build()
    nc = _CACHE["nc"]

    def lay_kd(a):  # [e|h, cols] -> [p, k, d, cols] -> [128, -1]
        n = a.shape[1]
        return np.ascontiguousarray(
            a.reshape(8, 2, 128, n).transpose(2, 0, 1, 3).reshape(128, -1))

    s4_hi, s4_lo = _split(out_state.T, 4.0, e4)      # [e, i_global]
    w_hi, w_lo = _split(W, 64.0, e4)                 # [e, h]
    h4_hi, h4_lo = _split(history.T, 4.0, e4)        # [h, j]

    def lay_h(a):  # [h, j] -> [p, s, q, d, 512] -> [128, -1]
        b = a.reshape(HP, 2, 128, SEQ).transpose(2, 0, 1, 3)   # [p, q, d, j]
        b = b.reshape(128, HP, 2, NSLAB, 512).transpose(0, 3, 1, 2, 4)
        b = np.ascontiguousarray(b.reshape(128, NSLAB, HP, 2, 512))
        flat = b[:, :NSLAB - 1].reshape(128, -1)
        # slab 7 split into a 384-col then a 128-col piece
        b7 = b[:, NSLAB - 1]                       # [p, q, d, 512]
        pa = b7[..., :384].reshape(128, -1)
        pb = b7[..., 384:].reshape(128, -1)
        return np.ascontiguousarray(np.concatenate([flat, pa, pb], axis=1))

    wm_hi, wm_lo = lay_kd(w_hi), lay_kd(w_lo)
    hm_hi, hm_lo = lay_h(h4_hi), lay_h(h4_lo)

    in_maps = []
    for c in range(NCORES):
        cs = slice(c * IS, (c + 1) * IS)
        in_maps.append({
            "s_hi": lay_kd(s4_hi[:, cs]),
            "s_lo": lay_kd(s4_lo[:, cs]),
            "w_hi": wm_hi, "w_lo": wm_lo,
            "h_hi": hm_hi, "h_lo": hm_lo,
        })
    res = run_bass_kernel_spmd(nc, in_maps, core_ids=list(range(NCORES)),
                               trace=TRACE)
    _CACHE["last_result"] = res
    return np.concatenate(
        [res.results[c]["out"] for c in range(NCORES)], axis=0
    ).astype(np.float32)
